# revision 15
# baseline (speedup 1.0000x reference)
"""DirSAGEConv Trainium2 kernel: 8-core SPMD gather + one-hot-matmul scatter.

out = x @ Ws.T + bs + (1-a)*(mean_{src->dst}(x) @ W1.T + b1)
                    + a*(mean_{dst->src}(x) @ W2.T + b2)

Sharding: nodes (and the messages that scatter into them) are split into 8
contiguous blocks of 12500, one per NeuronCore. The x table (bf16) is
replicated in every core's DRAM; each core dma_gathers the source rows of
its ~400k messages (int16 indices, two 64k-row segments via signed offsets),
and the TensorEngine scatter-accumulates them with per-message weight
matrices (fp8e3 one-hot columns scaled by 8/deg, so the mean division is
folded into the scatter). The PSUM accumulator is kept transposed
[feat x slot], so it feeds the three 128x128 weight matmuls directly with
no extra transpose; the 1/8 is folded into W1/W2 on the host.
"""

import math
import sys
import types

import numpy as np

try:
    import ml_dtypes
except ImportError:  # pragma: no cover
    ml_dtypes = None

import concourse.bacc as bacc
import concourse.bass as bass
import concourse.mybir as mybir
import concourse.tile as tile
from concourse.bass_utils import run_bass_kernel_spmd

ALPHA = 0.5
NCORES = 8
P = 128          # partitions / feature dim / window rows / tile msgs
SEG_ROWS = 32768  # int16 gather index reach
BATCH_W = 7      # windows per batch (7 acc PSUM banks + 1 out bank)
MAX_CALL_IDXS = 1024  # per dma_gather call (single_packet: <=64 descs/engine)
import os as _os
_SP = _os.environ.get("GATHER_SP", "1") == "1"
_MC = int(_os.environ.get("GATHER_MAXCALL", "0")) or None

BF16 = np.dtype(ml_dtypes.bfloat16) if ml_dtypes is not None else None
FP8 = np.dtype(ml_dtypes.float8_e3m4) if ml_dtypes is not None else None


def _install_profile_hook():
    """Wire the NTFF profile hook trn_boot would install if antenv had
    axon_hooks (needed for trace=True exec_time_ns under axon)."""
    import antenv

    try:
        from antenv import axon_hooks  # noqa: F401

        return
    except ImportError:
        pass
    m = types.ModuleType("antenv.axon_hooks")
    m._hook = None
    m.set_axon_ntff_profile_hook = lambda h: setattr(m, "_hook", h)
    m.get_axon_ntff_profile_hook = lambda: m._hook
    sys.modules["antenv.axon_hooks"] = m
    antenv.axon_hooks = m
    try:
        if "/root/.axon_site" not in sys.path:
            sys.path.insert(0, "/root/.axon_site")
        from trn_agent_boot import trn_boot

        hook = trn_boot._ntff_profile_via_ctypes("/opt/axon/libaxon_pjrt.so")
        m.set_axon_ntff_profile_hook(hook)
    except Exception:
        pass


class Plan:
    """Static (core-uniform) message layout + per-core data arrays."""


def make_plan(edge_index, n_nodes, n_cores=NCORES, seg_rows=SEG_ROWS,
              batch_w=BATCH_W):
    pl = Plan()
    npc = n_nodes // n_cores
    assert npc * n_cores == n_nodes
    nw = (npc + P - 1) // P           # scatter windows per core
    nseg = (n_nodes + seg_rows - 1) // seg_rows
    pl.n_nodes, pl.n_cores, pl.npc, pl.nw, pl.nseg = n_nodes, n_cores, npc, nw, nseg
    pl.seg_rows = seg_rows
    pl.xrows = max(n_nodes, (n_cores - 1) * npc + nw * P)
    pl.xrows = (pl.xrows + P - 1) // P * P

    src = np.ascontiguousarray(edge_index[0]).astype(np.int64)
    dst = np.ascontiguousarray(edge_index[1]).astype(np.int64)
    # direction 0: gather src, scatter dst (m_s2d); direction 1: the reverse
    g = np.concatenate([src, dst])
    s = np.concatenate([dst, src])
    d = np.repeat(np.array([0, 1], np.int64), src.shape[0])

    owner = s // npc
    sl = s - owner * npc
    win = sl >> 7
    soff = (sl & 127).astype(np.float32)
    seg = g // seg_rows
    gloc = (g - seg * seg_rows).astype(np.int16)
    cell = ((owner * 2 + d) * nw + win) * nseg + seg

    order = np.lexsort((g, cell))
    cell_s = cell[order]
    gloc_s = gloc[order]
    soff_s = soff[order]
    # per-message scatter weight: 8/deg(dst) (the /8 is folded into W1/W2)
    slot = ((owner * 2 + d) * nw + win) * P + (sl & 127)
    cntv = np.bincount(slot, minlength=n_cores * 2 * nw * P)
    wv_all = (8.0 / np.maximum(cntv[slot], 1)).astype(np.float32)
    wv_s = wv_all[order]

    ncells = n_cores * 2 * nw * nseg
    counts = np.bincount(cell_s, minlength=ncells)
    cum = np.zeros(ncells + 1, np.int64)
    np.cumsum(counts, out=cum[1:])
    # tiles per (d, w, seg): max over cores (SPMD uniform shapes)
    T = np.ceil(counts.reshape(n_cores, 2, nw, nseg) / P).astype(np.int64).max(axis=0)
    pl.T = T

    batches = [list(range(i, min(i + batch_w, nw))) for i in range(0, nw, batch_w)]
    pl.batches = batches

    # enumerate the flat slot/tile stream (identical for every core)
    cell_slot = {}
    calls = {}        # (d,b,seg) -> [(w, slot_off, n_slots), ...] per cell
    segext = {}       # (d,b,seg) -> (seg_slot0, seg_nslots)
    binfo = {}        # (d,b) -> dict
    slot_off = 0
    tile_off = 0
    for di in range(2):
        for bi, wins in enumerate(batches):
            b0s, b0t = slot_off, tile_off
            for sg in range(nseg):
                c0 = slot_off
                cl = []
                for w in wins:
                    t = int(T[di, w, sg])
                    if t:
                        cell_slot[(di, w, sg)] = slot_off
                        cl.append((w, slot_off, t * P))
                        slot_off += t * P
                        tile_off += t
                if slot_off > c0:
                    calls[(di, bi, sg)] = cl
                    segext[(di, bi, sg)] = (c0, slot_off - c0)
            binfo[(di, bi)] = dict(slot0=b0s, slot1=slot_off, tile0=b0t,
                                   tile1=tile_off)
    pl.segext = segext
    pl.cell_slot, pl.calls, pl.binfo = cell_slot, calls, binfo
    # per-core real count for every gather call, in emission order
    call_order = []
    for di in range(2):
        for bi in range(len(batches)):
            for sg in range(nseg):
                if (di, bi, sg) in calls:
                    for (w, off, n) in calls[(di, bi, sg)]:
                        call_order.append((di, w, sg))
    pl.ncalls = len(call_order)
    call_counts = np.zeros((n_cores, pl.ncalls), np.uint32)
    for k, (di, w, sg) in enumerate(call_order):
        for c in range(n_cores):
            cid = ((c * 2 + di) * nw + w) * nseg + sg
            call_counts[c, k] = counts[cid]
    pl.call_counts = call_counts
    pl.total_slots, pl.total_tiles = slot_off, tile_off
    pl.tiles_per_win = T.sum(axis=2)   # [2, nw]

    # per-core padded streams (pad idx = -1: trailing pads per cell are
    # dropped by the gather ucode, per core)
    gidx = np.full((n_cores, slot_off), -1, np.int16)
    sval = np.full((n_cores, slot_off), -1.0, np.float32)
    wval = np.zeros((n_cores, slot_off), np.float32)
    for c in range(n_cores):
        for (di, w, sg), off in cell_slot.items():
            cid = ((c * 2 + di) * nw + w) * nseg + sg
            a, b = cum[cid], cum[cid + 1]
            n = b - a
            if n:
                gidx[c, off:off + n] = gloc_s[a:b]
                sval[c, off:off + n] = soff_s[a:b]
                wval[c, off:off + n] = wv_s[a:b]

    # dma_gather index layout: idx j of a call -> [j % 16, j // 16], tiled x8
    gidx_dram = np.zeros((n_cores, P, slot_off // 16), np.int16)
    for cl in calls.values():
        for (_, off, n) in cl:
            blk = gidx[:, off:off + n].reshape(n_cores, n // 16, 16)
            blk = blk.transpose(0, 2, 1)                   # [C, 16, n/16]
            gidx_dram[:, :, off // 16:(off + n) // 16] = np.tile(blk, (1, 8, 1))
    pl.gidx_dram = gidx_dram
    # host-built scatter tiles: oh[c][m, t*P+s] = wv (at s = sval), 0 else
    lut = np.zeros((P + 1, P), np.float32)
    lut[1:] = np.eye(P, dtype=np.float32)
    idx = (sval.reshape(n_cores, tile_off, P).astype(np.int64) + 1)
    oh_dram = np.empty((n_cores, P, tile_off * P), FP8)
    for c in range(n_cores):
        ohc = lut[idx[c]] * wval[c].reshape(tile_off, P, 1)  # [T, m, s]
        oh_dram[c] = np.ascontiguousarray(
            ohc.transpose(1, 0, 2).reshape(P, tile_off * P)).astype(FP8)
    pl.oh_dram = oh_dram
    return pl


def build_program(pl, debug=False):
    dt = mybir.dt
    nc = bacc.Bacc("TRN2", target_bir_lowering=False, debug=debug,
                   num_devices=pl.n_cores, num_swdge_queues=4,
                   dynamic_dma_scratch_size=32768)
    nw, nseg = pl.nw, pl.nseg
    xg = nc.dram_tensor("xg", [pl.xrows, P], dt.bfloat16, kind="ExternalInput")
    xb = nc.dram_tensor("xb", [nw * P, P], dt.bfloat16, kind="ExternalInput")
    gi = nc.dram_tensor("gi", [P, pl.total_slots // 16], dt.int16,
                        kind="ExternalInput")
    ohd = nc.dram_tensor("ohd", [P, pl.total_tiles * P], dt.float8e3,
                         kind="ExternalInput")
    onesrow = nc.dram_tensor("onesrow", [1, P], dt.bfloat16, kind="ExternalInput")
    wst = nc.dram_tensor("wst", [P, P], dt.bfloat16, kind="ExternalInput")
    w1t = nc.dram_tensor("w1t", [P, P], dt.bfloat16, kind="ExternalInput")
    w2t = nc.dram_tensor("w2t", [P, P], dt.bfloat16, kind="ExternalInput")
    btot = nc.dram_tensor("btot", [1, P], dt.bfloat16, kind="ExternalInput")
    gcnt = nc.dram_tensor("gcnt", [1, pl.ncalls], dt.uint32,
                          kind="ExternalInput")
    outd = nc.dram_tensor("out", [P, nw, P], dt.float32, kind="ExternalOutput")

    qrr = [0]

    with tile.TileContext(nc) as tc:
        with (
            tc.tile_pool(name="const", bufs=1) as cpool,
            tc.tile_pool(name="gpool", bufs=2) as gpool,
            tc.tile_pool(name="ipool", bufs=3) as ipool,
            tc.tile_pool(name="sm", bufs=4) as smpool,
            tc.tile_pool(name="ob", bufs=2) as obpool,
            tc.tile_pool(name="acc", bufs=BATCH_W, space="PSUM") as accpool,
            tc.tile_pool(name="op", bufs=1, space="PSUM") as oppool,
        ):
            def cld(name, handle, shape):
                t = cpool.tile(shape, dt.bfloat16, tag=name)
                nc.sync.dma_start(t[:], handle[:])
                return t

            onesrow_t = cld("onesrow", onesrow, [1, P])
            wst_t = cld("wst", wst, [P, P])
            w1t_t = cld("w1t", w1t, [P, P])
            w2t_t = cld("w2t", w2t, [P, P])
            btot_t = cld("btot", btot, [1, P])
            gcnt_t = cpool.tile([1, pl.ncalls], dt.uint32, tag="gcnt")
            nc.sync.dma_start(gcnt_t[:], gcnt[:])
            kcall = [0]
            nregs = [nc.alloc_register(mybir.EngineType.Pool, f"nr{i}")
                     for i in range(16)]
            # zero-fill every gather-buffer generation once: slots trimmed by
            # the per-core dynamic count must stay finite (0*oh_pad==0)
            for sg in range(nseg):
                mx = max((pl.segext[k][1] for k in pl.segext if k[2] == sg),
                         default=0)
                if mx:
                    for _ in range(2):
                        gz = gpool.tile([P, mx // P, P], dt.bfloat16,
                                        tag=f"g{sg}")
                        nc.vector.memset(gz[:], 0.0)
            # dir-0 aggregate kept on-chip, [feat x slot] per window
            m1sb = cpool.tile([P, nw * P], dt.bfloat16, tag="m1sb")
            # whole transposed self-term block, loaded once
            xtall = cpool.tile([P, nw * P], dt.bfloat16, tag="xtall")
            nc.sync.dma_start(xtall[:], xb[:], transpose=True)

            for di in range(2):
                for bi, wins in enumerate(pl.batches):
                    info = pl.binfo[(di, bi)]
                    s0, s1 = info["slot0"], info["slot1"]
                    t0, t1 = info["tile0"], info["tile1"]
                    if s1 > s0:
                        it = ipool.tile([P, (s1 - s0) // 16], dt.int16, tag="gidx")
                        nc.sync.dma_start(it[:], gi[:, s0 // 16:s1 // 16])
                        oht = ipool.tile([P, (t1 - t0) * P], dt.float8e3,
                                         tag="oh")
                        nc.scalar.dma_start(oht[:], ohd[:, t0 * P:t1 * P])
                    gts = {}
                    for sg in range(nseg):
                        if (di, bi, sg) not in pl.calls:
                            continue
                        soff, sn = pl.segext[(di, bi, sg)]
                        gt = gpool.tile([P, sn // P, P], dt.bfloat16,
                                        tag=f"g{sg}")
                        a = sg * pl.seg_rows
                        b = min(a + pl.seg_rows, pl.xrows)
                        mci = _MC or MAX_CALL_IDXS
                        for (_, off, n) in pl.calls[(di, bi, sg)]:
                            k = kcall[0]
                            if k % 16 == 0:
                                ke = min(k + 16, pl.ncalls)
                                nc.gpsimd.reg_load(nregs[:ke - k],
                                                   gcnt_t[0:1, k:ke])
                            kcall[0] += 1
                            for c0 in range(0, n, mci):
                                cn = min(mci, n - c0)
                                g0 = off + c0 - soff
                                idx_ap = it[:, (off + c0 - s0) // 16:
                                            (off + c0 + cn - s0) // 16]
                                nc.gpsimd.dma_gather(
                                    gt[:, g0 // P:(g0 + cn) // P, :],
                                    xg[a:b, :],
                                    idx_ap, cn, nregs[k % 16], P, single_packet=_SP,
                                    queue_num=qrr[0] % 4)
                                qrr[0] += 1
                        gts[sg] = gt

                    accs = {w: accpool.tile([P, P], dt.float32, tag="acc",
                                            name=f"acc{w}")
                            for w in wins}
                    first = {w: True for w in wins}
                    left = {w: int(pl.tiles_per_win[di, w]) for w in wins}
                    # emit matmuls in stream-tile order; lhsT = gathered rows,
                    # rhs = weighted one-hot -> acc[feat, slot] (transposed)
                    gtile = t0
                    for sg in range(nseg):
                        if sg not in gts:
                            continue
                        tl = 0
                        for w in wins:
                            acc = accs[w]
                            for _ in range(int(pl.T[di, w, sg])):
                                ps = oht[:, (gtile - t0) * P:(gtile - t0 + 1) * P]
                                left[w] -= 1
                                nc.tensor.matmul(
                                    acc[:, 0:P], lhsT=gts[sg][:, tl, :],
                                    rhs=ps,
                                    start=first[w], stop=(left[w] == 0))
                                first[w] = False
                                tl += 1
                                gtile += 1
                    # drain the batch's windows
                    cp = mybir.ActivationFunctionType.Copy
                    if di == 1:
                        obb = obpool.tile([P, len(wins) * P], dt.float32,
                                          tag="ob")
                    for w in wins:
                        acc = accs[w]
                        if pl.tiles_per_win[di, w] == 0:
                            nc.vector.memset(acc[:, 0:P], 0.0)
                        if di == 0:
                            nc.scalar.activation(m1sb[:, w * P:(w + 1) * P],
                                                 acc[:, 0:P], cp)
                        else:
                            mt = smpool.tile([P, P], dt.bfloat16, tag="mt")
                            nc.scalar.activation(mt[:], acc[:, 0:P], cp)
                            op = oppool.tile([P, P], dt.float32, tag="op")
                            nc.tensor.matmul(op[:], lhsT=xtall[:, w * P:(w + 1) * P],
                                             rhs=wst_t[:], start=True, stop=False)
                            nc.tensor.matmul(op[:], lhsT=m1sb[:, w * P:(w + 1) * P],
                                             rhs=w1t_t[:], start=False, stop=False)
                            nc.tensor.matmul(op[:], lhsT=mt[:], rhs=w2t_t[:],
                                             start=False, stop=False)
                            nc.tensor.matmul(op[:], lhsT=onesrow_t[:],
                                             rhs=btot_t[:], start=False, stop=True)
                            k = w - wins[0]
                            nc.scalar.activation(obb[:, k * P:(k + 1) * P],
                                                 op[:], cp)
                    if di == 1:
                        nc.sync.dma_start(
                            outd[:, wins[0]:wins[0] + len(wins), :], obb[:])

    nc.compile()
    return nc


def make_inputs(pl, x, W1, b1, W2, b2, Ws, bs):
    """Per-core in_maps from the full inputs."""
    bf = BF16
    xpad = np.zeros((pl.xrows, P), np.float32)
    xpad[:pl.n_nodes] = np.asarray(x, np.float32)
    xg = xpad.astype(bf)
    onesrow = np.ones((1, P), np.float32).astype(bf)
    wst = np.ascontiguousarray(np.asarray(Ws, np.float32).T).astype(bf)
    w1t = np.ascontiguousarray(
        (1.0 - ALPHA) / 8.0 * np.asarray(W1, np.float32).T).astype(bf)
    w2t = np.ascontiguousarray(
        ALPHA / 8.0 * np.asarray(W2, np.float32).T).astype(bf)
    btot = (np.asarray(bs, np.float32) + (1.0 - ALPHA) * np.asarray(b1, np.float32)
            + ALPHA * np.asarray(b2, np.float32)).reshape(1, P).astype(bf)
    in_maps = []
    for c in range(pl.n_cores):
        in_maps.append({
            "xg": xg,
            "xb": np.ascontiguousarray(xg[c * pl.npc:c * pl.npc + pl.nw * P]),
            "gi": np.ascontiguousarray(pl.gidx_dram[c]),
            "ohd": pl.oh_dram[c],
            "onesrow": onesrow,
            "gcnt": np.ascontiguousarray(pl.call_counts[c].reshape(1, -1)),
            "wst": wst, "w1t": w1t, "w2t": w2t, "btot": btot,
        })
    return in_maps


def kernel(x, edge_index, W1, b1, W2, b2, Ws, bs, _trace=False):
    x = np.asarray(x)
    n_nodes = x.shape[0]
    pl = make_plan(np.asarray(edge_index), n_nodes)
    nc = build_program(pl)
    in_maps = make_inputs(pl, x, W1, b1, W2, b2, Ws, bs)
    if _trace:
        _install_profile_hook()
    import os

    res = run_bass_kernel_spmd(nc, in_maps, core_ids=list(range(pl.n_cores)),
                               trace=_trace,
                               tmpdir=os.environ.get("BASS_TMPDIR") or None)
    out = np.empty((n_nodes, P), np.float32)
    for c in range(pl.n_cores):
        oc = res.results[c]["out"].transpose(1, 0, 2).reshape(pl.nw * P, P)
        out[c * pl.npc:(c + 1) * pl.npc] = oc[:pl.npc]
    if _trace:
        kernel._last_exec_ns = res.exec_time_ns
        kernel._last_results = res
    return out
